# revision 3
# baseline (speedup 1.0000x reference)
"""Trainium2 Bass kernel for nn_Net_75230647157021 (moe_routing).

Pure data parallel over 8 NeuronCores: batch 262144 -> 8 shards of 32768;
parameters replicated. One SPMD Bass module via run_bass_kernel_spmd.

On-device layout is feature-major: activations are [D, F] tiles (features on
partitions, samples on the free axis); every linear is one PE matmul
(lhsT = [Din, Dout], rhs = activations, N = 512 per PSUM bank). A hardware
For_i loop runs 16 super-tiles of 4x512 = 2048 samples per core.

LayerNorm: mean-subtraction is folded into the preceding matmul on the host
(W <- W - rowmean(W), b <- b - mean(b)) so the matmul output is already
centered and variance = mean(z^2). Per 512-tile: evict+bias (DVE), square
(GpSimd), column-sum matmul (PE, M=1, col-group packed 4 tiles into one
PSUM bank), rsqrt via DVE bit-trick + 2 bf16 Newton steps on the packed
[128,512] stat tile, replicate-matmul (PE, K=1, lhsT = gamma*sqrt(D) packed
at rows 0/32/64/96), normalize multiply (DVE), one ACT Silu with
per-partition beta bias. ACT Rsqrt/Reciprocal are banned (accuracy), and
Exp/Ln live in a different ACT table set than Silu, so the gate softmax
normalization uses a DVE bit-trick reciprocal.

Device output rows are {strain, tensile, softplus(gap)} as [3, 32768] fp32
per core; the host computes yield = tensile - softplus and assembles
[262144, 3].
"""

import sys
import numpy as np
from contextlib import ExitStack

sys.path.insert(0, "/opt/trn_rl_repo")

import concourse.bacc as bacc
import concourse.tile as tile
from concourse import mybir
from concourse.bass_utils import run_bass_kernel_spmd
import concourse.bass as bass

F32 = mybir.dt.float32
BF16 = mybir.dt.bfloat16
U32 = mybir.dt.uint32
AF = mybir.ActivationFunctionType
ALU = mybir.AluOpType

N_CORES = 8
B_TOTAL = 262144
B_CORE = B_TOTAL // N_CORES       # 32768
FD = 512                          # samples per PSUM tile
NT = 4                            # sub-tiles per super-tile
SUP = NT * FD                     # 2048 samples per super-tile
N_SUP = B_CORE // SUP             # 16 super-tiles per core
EPS = 1e-5
RSQRT_MAGIC = 0x5F3759DF
RECIP_MAGIC = 0x7EF477D5

_cached = {}


def _bf16(a):
    import ml_dtypes
    return np.ascontiguousarray(np.asarray(a, dtype=np.float32).astype(ml_dtypes.bfloat16))


def _col(a):
    return np.ascontiguousarray(np.asarray(a, np.float32).reshape(-1, 1))


def _prep_params(params):
    def _conv(v):
        if isinstance(v, dict):
            return {k: _conv(vv) for k, vv in v.items()}
        return np.asarray(v, np.float64)

    p = _conv(dict(params))
    ins = {}

    def ln_lin(name, w, b, g, beta):
        D = w.shape[1]
        wc = w - w.mean(axis=1, keepdims=True)
        bc = b - b.mean()
        ins[name + "_w"] = _bf16(wc)
        ins[name + "_bc"] = _col(bc)
        grp = np.zeros((128, D))
        for j in range(NT):
            grp[32 * j, :] = g * np.sqrt(D)
        ins[name + "_gr"] = _bf16(grp)
        ins[name + "_beta"] = _col(beta)

    def lin(name, w, b):
        ins[name + "_w"] = _bf16(w)
        ins[name + "_b"] = _col(b)

    ln_lin("stem1", p["stem_l1"]["w"], p["stem_l1"]["b"],
           p["stem_ln1"]["g"], p["stem_ln1"]["b"])
    ln_lin("stem2", p["stem_l2"]["w"], p["stem_l2"]["b"],
           p["stem_ln2"]["g"], p["stem_ln2"]["b"])
    for bi, bn in enumerate(["block1", "block2"]):
        blk = p[bn]
        ln_lin(f"b{bi}f1", blk["fc1"]["w"], blk["fc1"]["b"],
               blk["ln1"]["g"], blk["ln1"]["b"])
        ln_lin(f"b{bi}f2", blk["fc2"]["w"], blk["fc2"]["b"],
               blk["ln2"]["g"], blk["ln2"]["b"])
    lin("g1", p["gate_l1"]["w"], p["gate_l1"]["b"])
    lin("g2", p["gate_l2"]["w"], p["gate_l2"]["b"])
    e = p["experts"]
    for ei in range(6):
        ln_lin(f"e{ei}w1", e["w1"][ei], e["b1"][ei], e["lng"][ei], e["lnb"][ei])
        lin(f"e{ei}w2", e["w2"][ei], e["b2"][ei])
        lin(f"e{ei}w3", e["w3"][ei], e["b3"][ei])
    ln_lin("post", p["post_l"]["w"], p["post_l"]["b"],
           p["post_ln"]["g"], p["post_ln"]["b"])
    lin("shared", p["shared"]["w"], p["shared"]["b"])

    ins["g3_w"] = _bf16(p["gate_l3"]["w"])
    g3b = np.zeros((128, 1), np.float32)
    for j in range(NT):
        g3b[32 * j:32 * j + 6, 0] = p["gate_l3"]["b"]
    ins["g3_bpack"] = g3b

    hs, ht, hg = p["head_strain"], p["head_tensile"], p["yield_gap"]
    ins["h1a_w"] = _bf16(np.concatenate([hs["l1"]["w"], ht["l1"]["w"]], axis=1))
    ins["h1a_b"] = _col(np.concatenate([hs["l1"]["b"], ht["l1"]["b"]]))
    ins["h1b_w"] = _bf16(hg["l1"]["w"])
    ins["h1b_b"] = _col(hg["l1"]["b"])
    l2a = np.zeros((128, 64))
    l2a[:64, :32] = hs["l2"]["w"]
    l2a[64:, 32:] = ht["l2"]["w"]
    ins["h2a_w"] = _bf16(l2a)
    ins["h2b_w"] = _bf16(hg["l2"]["w"])
    ins["h2_b"] = _col(np.concatenate([hs["l2"]["b"], ht["l2"]["b"],
                                       hg["l2"]["b"]]))
    l3 = np.zeros((96, 3))
    l3[:32, 0] = hs["l3"]["w"][:, 0]
    l3[32:64, 1] = ht["l3"]["w"][:, 0]
    l3[64:, 2] = hg["l3"]["w"][:, 0]
    ins["h3_w"] = _bf16(l3)
    h3b = np.zeros((128, 1), np.float32)
    for j in range(NT):
        h3b[32 * j + 0, 0] = hs["l3"]["b"][0]
        h3b[32 * j + 1, 0] = ht["l3"]["b"][0]
        h3b[32 * j + 2, 0] = hg["l3"]["b"][0]
    ins["h3_bpack"] = h3b

    ins["ones"] = _bf16(np.ones((128, 128)))
    return ins


def _build(param_shapes):
    nc = bacc.Bacc()
    x_d = nc.dram_tensor("x", [64, B_CORE], BF16, kind="ExternalInput")
    out_d = nc.dram_tensor("out", [3, B_CORE], F32, kind="ExternalOutput")
    w_d = {}
    for name, arr in param_shapes.items():
        dt = BF16 if arr.dtype != np.float32 else F32
        w_d[name] = nc.dram_tensor(name, list(arr.shape), dt, kind="ExternalInput")

    with tile.TileContext(nc) as tc, ExitStack() as ctx:
        wp = ctx.enter_context(tc.tile_pool(name="wp", bufs=1))
        a1 = ctx.enter_context(tc.tile_pool(name="a1", bufs=1))
        a2 = ctx.enter_context(tc.tile_pool(name="a2", bufs=2))
        a3 = ctx.enter_context(tc.tile_pool(name="a3", bufs=3))
        stat = ctx.enter_context(tc.tile_pool(name="stat", bufs=1))
        zp = ctx.enter_context(tc.tile_pool(name="zp", bufs=2, space="PSUM"))
        rp = ctx.enter_context(tc.tile_pool(name="rp", bufs=2, space="PSUM"))
        vp = ctx.enter_context(tc.tile_pool(name="vp", bufs=2, space="PSUM"))
        mp = ctx.enter_context(tc.tile_pool(name="mp", bufs=2, space="PSUM"))

        W = {}
        for name, arr in param_shapes.items():
            dt = BF16 if arr.dtype != np.float32 else F32
            t = wp.tile(list(arr.shape), dt, tag=name)
            nc.sync.dma_start(out=t[:], in_=w_d[name][:])
            W[name] = t

        JS = [slice(j * FD, (j + 1) * FD) for j in range(NT)]

        def rsqrt_chain(vpack_ps, d_eps):
            ve = stat.tile([128, FD], F32, tag="ve")
            nc.vector.tensor_scalar(ve[:], vpack_ps[:], float(d_eps), None,
                                    ALU.add)
            i1 = stat.tile([128, FD], U32, tag="i1")
            nc.vector.tensor_scalar(i1[:], ve[:].bitcast(U32), 1, None,
                                    ALU.logical_shift_right)
            sd = stat.tile([128, FD], F32, tag="sd")
            nc.vector.tensor_scalar(sd[:].bitcast(U32), i1[:], -1, RSQRT_MAGIC,
                                    ALU.mult, ALU.add)
            sdb = stat.tile([128, FD], BF16, tag="sdb")
            nc.vector.tensor_copy(sdb[:], sd[:])
            web = stat.tile([128, FD], BF16, tag="web")
            nc.vector.tensor_copy(web[:], ve[:])
            cur = sdb
            for it in range(2):
                a = stat.tile([128, FD], BF16, tag=f"nA{it}")
                nc.vector.tensor_tensor(a[:], cur[:], cur[:], ALU.mult)
                b = stat.tile([128, FD], BF16, tag=f"nB{it}")
                nc.vector.tensor_tensor(b[:], a[:], web[:], ALU.mult)
                c = stat.tile([128, FD], BF16, tag=f"nC{it}")
                nc.vector.tensor_scalar(c[:], b[:], -0.5, 1.5, ALU.mult,
                                        ALU.add)
                r = stat.tile([128, FD], BF16, tag=f"nR{it}")
                nc.vector.tensor_tensor(r[:], cur[:], c[:], ALU.mult)
                cur = r
            return cur

        def recip_chain(spack_ps):
            sd = stat.tile([128, FD], F32, tag="rsd")
            nc.vector.tensor_scalar(sd[:].bitcast(U32), spack_ps[:].bitcast(U32),
                                    -1, RECIP_MAGIC, ALU.mult, ALU.add)
            sb = stat.tile([128, FD], BF16, tag="rsb")
            nc.vector.tensor_copy(sb[:], sd[:])
            wb = stat.tile([128, FD], BF16, tag="rwb")
            nc.vector.tensor_copy(wb[:], spack_ps[:])
            cur = sb
            for it in range(2):
                d = stat.tile([128, FD], BF16, tag=f"rD{it}")
                nc.vector.tensor_tensor(d[:], wb[:], cur[:], ALU.mult)
                e = stat.tile([128, FD], BF16, tag=f"rE{it}")
                nc.vector.tensor_scalar(e[:], d[:], -1.0, 2.0, ALU.mult,
                                        ALU.add)
                r = stat.tile([128, FD], BF16, tag=f"rR{it}")
                nc.vector.tensor_tensor(r[:], cur[:], e[:], ALU.mult)
                cur = r
            return cur

        def ln_layer(name, insup, din, dout, opool, otag, res=None):
            w, bc = W[name + "_w"], W[name + "_bc"]
            gr, beta = W[name + "_gr"], W[name + "_beta"]
            zsup = a2.tile([dout, SUP], BF16, tag="ln_z")
            sqsup = a2.tile([dout, SUP], BF16, tag="ln_sq")
            vpack = vp.tile([128, FD], F32, tag="vpack")
            for j in range(NT):
                z = zp.tile([128, FD], F32, tag="z")
                nc.tensor.matmul(z[:dout, :], w[:, :], insup[:, JS[j]],
                                 start=True, stop=True)
                nc.vector.tensor_scalar(zsup[:, JS[j]], z[:dout, :], bc[:, :],
                                        None, ALU.add)
                nc.gpsimd.tensor_tensor(sqsup[:, JS[j]], zsup[:, JS[j]],
                                        zsup[:, JS[j]], ALU.mult)
                nc.tensor.matmul(vpack[32 * j:32 * j + 1, :],
                                 W["ones"][:dout, 0:1], sqsup[:, JS[j]],
                                 start=True, stop=True,
                                 tile_position=(0, 32 * j))
            inv = rsqrt_chain(vpack, dout * EPS)
            tsup = a2.tile([dout, SUP], BF16, tag="ln_t")
            for j in range(NT):
                ir = rp.tile([128, FD], F32, tag="ir")
                nc.tensor.matmul(ir[:dout, :], gr[32 * j:32 * j + 1, :],
                                 inv[32 * j:32 * j + 1, :],
                                 start=True, stop=True,
                                 tile_position=(32 * j, 0))
                nc.vector.tensor_tensor(tsup[:, JS[j]], zsup[:, JS[j]],
                                        ir[:dout, :], ALU.mult)
            outsup = opool.tile([dout, SUP], BF16, tag=otag)
            if res is None:
                nc.scalar.activation(outsup[:], tsup[:], AF.Silu,
                                     bias=beta[:, :])
            else:
                rsup = a1.tile([dout, SUP], BF16, tag="ln_r")
                nc.gpsimd.tensor_tensor(rsup[:], tsup[:], res[:], ALU.add)
                nc.scalar.activation(outsup[:], rsup[:], AF.Silu,
                                     bias=beta[:, :])
            return outsup

        def lin_silu(name, insup, din, dout, opool, otag):
            w, b = W[name + "_w"], W[name + "_b"]
            outsup = opool.tile([dout, SUP], BF16, tag=otag)
            for j in range(NT):
                z = zp.tile([128, FD], F32, tag="z")
                nc.tensor.matmul(z[:dout, :], w[:, :], insup[:, JS[j]],
                                 start=True, stop=True)
                nc.scalar.activation(outsup[:, JS[j]], z[:dout, :], AF.Silu,
                                     bias=b[:, :])
            return outsup

        def body(iv):
            xoff = iv * SUP
            xs = a2.tile([64, SUP], BF16, tag="xs")
            nc.sync.dma_start(out=xs[:], in_=x_d[:, bass.ds(xoff, SUP)])

            t = ln_layer("stem1", xs, 64, 128, a3, "ln_o")
            h0 = ln_layer("stem2", t, 128, 96, a1, "hres0")
            t = ln_layer("b0f1", h0, 96, 128, a3, "ln_o")
            h1 = ln_layer("b0f2", t, 128, 96, a1, "hres1", res=h0)
            t = ln_layer("b1f1", h1, 96, 128, a3, "ln_o")
            h = ln_layer("b1f2", t, 128, 96, a1, "h", res=h1)

            # ---- gate ----
            g1 = lin_silu("g1", h, 96, 64, a1, "g1_o")
            g2 = lin_silu("g2", g1, 64, 32, a1, "g2_o")
            logit = mp.tile([128, FD], F32, tag="mp")
            nc.vector.memset(logit[:], 0.0)
            for j in range(NT):
                nc.tensor.matmul(logit[32 * j:32 * j + 6, :], W["g3_w"][:, :],
                                 g2[:, JS[j]], start=True, stop=True,
                                 tile_position=(0, 32 * j))
            expp = a1.tile([128, FD], BF16, tag="expp")
            nc.scalar.activation(expp[:], logit[:], AF.Exp,
                                 bias=W["g3_bpack"][:, :])
            spack = mp.tile([128, FD], F32, tag="mp")
            nc.vector.memset(spack[:], 0.0)
            for j in range(NT):
                nc.tensor.matmul(spack[32 * j:32 * j + 1, :],
                                 W["ones"][32 * j:32 * j + 6, 0:1],
                                 expp[32 * j:32 * j + 6, :],
                                 start=True, stop=True,
                                 tile_position=(32 * j, 32 * j))
            rcp = recip_chain(spack)
            # repack exp rows so each expert's gate row sits 32-aligned
            epks = []
            for ei in range(6):
                epk = a1.tile([128, FD], BF16, tag=f"epk{ei}")
                src = expp[:].rearrange("(a b) f -> a b f", b=32)[:, ei, :]
                dst = epk[:].rearrange("(a b) f -> a b f", b=32)[:, 0, :]
                nc.sync.dma_start(out=dst, in_=src)
                epks.append(epk)

            # ---- experts + moe ----
            macc = a1.tile([96, SUP], BF16, tag="macc")
            for ei in range(6):
                u = ln_layer(f"e{ei}w1", h, 96, 96, a3, "ew_u")
                u = lin_silu(f"e{ei}w2", u, 96, 96, a3, "ew_u")
                u = lin_silu(f"e{ei}w3", u, 96, 96, a3, "ew_u")
                term = macc if ei == 0 else a2.tile([96, SUP], BF16, tag="mt")
                for j in range(NT):
                    gr_ps = rp.tile([128, FD], F32, tag="ir")
                    nc.tensor.matmul(gr_ps[:96, :],
                                     W["ones"][32 * j:32 * j + 1, :96],
                                     epks[ei][32 * j:32 * j + 1, :],
                                     start=True, stop=True,
                                     tile_position=(32 * j, 0))
                    nc.vector.tensor_tensor(term[:, JS[j]], u[:, JS[j]],
                                            gr_ps[:96, :], ALU.mult)
                if ei > 0:
                    nc.gpsimd.tensor_tensor(macc[:], macc[:], term[:], ALU.add)

            moe = a1.tile([96, SUP], BF16, tag="moe")
            for j in range(NT):
                rs_ps = rp.tile([128, FD], F32, tag="ir")
                nc.tensor.matmul(rs_ps[:96, :],
                                 W["ones"][32 * j:32 * j + 1, :96],
                                 rcp[32 * j:32 * j + 1, :],
                                 start=True, stop=True,
                                 tile_position=(32 * j, 0))
                nc.vector.tensor_tensor(moe[:, JS[j]], macc[:, JS[j]],
                                        rs_ps[:96, :], ALU.mult)
            pre = a3.tile([96, SUP], BF16, tag="ln_o")
            nc.gpsimd.tensor_tensor(pre[:], moe[:], h[:], ALU.add)

            # ---- post / shared / heads ----
            feat = ln_layer("post", pre, 96, 96, a3, "ln_o")
            sh = lin_silu("shared", feat, 96, 64, a1, "sh_o")
            h1a = lin_silu("h1a", sh, 64, 128, a1, "h1a_o")
            h1b = lin_silu("h1b", sh, 64, 64, a1, "h1b_o")
            hh2 = a1.tile([96, SUP], BF16, tag="hh2")
            for j in range(NT):
                z = zp.tile([128, FD], F32, tag="z")
                nc.tensor.matmul(z[:64, :], W["h2a_w"][:, :], h1a[:, JS[j]],
                                 start=True, stop=True)
                nc.tensor.matmul(z[64:96, :], W["h2b_w"][:, :], h1b[:, JS[j]],
                                 start=True, stop=True, tile_position=(0, 64))
                nc.scalar.activation(hh2[:, JS[j]], z[:96, :], AF.Silu,
                                     bias=W["h2_b"][:, :])
            l3p = mp.tile([128, FD], F32, tag="mp")
            nc.vector.memset(l3p[:], 0.0)
            for j in range(NT):
                nc.tensor.matmul(l3p[32 * j:32 * j + 3, :], W["h3_w"][:, :],
                                 hh2[:, JS[j]], start=True, stop=True,
                                 tile_position=(0, 32 * j))
            sb3 = stat.tile([128, FD], F32, tag="sb3")
            nc.vector.tensor_scalar(sb3[:], l3p[:], W["h3_bpack"][:, :],
                                    None, ALU.add)
            ex3 = stat.tile([128, FD], F32, tag="ex3")
            nc.scalar.activation(ex3[:], sb3[:], AF.Exp)
            sp3 = stat.tile([128, FD], F32, tag="sp3")
            nc.scalar.activation(sp3[:], ex3[:], AF.Ln, bias=1.0)
            for j in range(NT):
                nc.sync.dma_start(out=out_d[0:2, bass.ds(xoff + j * FD, FD)],
                                  in_=sb3[32 * j:32 * j + 2, :])
                nc.sync.dma_start(out=out_d[2:3, bass.ds(xoff + j * FD, FD)],
                                  in_=sp3[32 * j + 2:32 * j + 3, :])

        with tc.For_i(0, N_SUP, 1) as iv:
            body(iv)

    nc.finalize()
    return nc


def kernel(x, params):
    import ml_dtypes
    x = np.asarray(x, np.float32)
    ins = _prep_params(params)

    if "net" not in _cached:
        _cached["net"] = _build(ins)
    nc = _cached["net"]

    xT = np.ascontiguousarray(x.T).astype(ml_dtypes.bfloat16)  # [64, B]
    in_maps = []
    for c in range(N_CORES):
        m = dict(ins)
        m["x"] = np.ascontiguousarray(xT[:, c * B_CORE:(c + 1) * B_CORE])
        in_maps.append(m)

    res = run_bass_kernel_spmd(nc, in_maps, core_ids=list(range(N_CORES)),
                               trace=False)
    outs = []
    for c in range(N_CORES):
        o = res.results[c]["out"]
        y = np.empty((B_CORE, 3), np.float32)
        y[:, 0] = o[0]
        y[:, 1] = o[1]
        y[:, 2] = o[1] - o[2]
        outs.append(y)
    return np.concatenate(outs, axis=0)


# revision 4
# speedup vs baseline: 1.0918x; 1.0918x over previous
"""Trainium2 Bass kernel for nn_Net_75230647157021 (moe_routing).

Pure data parallel over 8 NeuronCores: batch 262144 -> 8 shards of 32768;
parameters replicated. One SPMD Bass module via run_bass_kernel_spmd.

On-device layout is feature-major: activations are [D, F] tiles (features on
partitions, samples on the free axis); every linear is one PE matmul
(lhsT = [Din, Dout], rhs = activations, N = 512 per PSUM bank). A hardware
For_i loop runs 16 super-tiles of 4x512 = 2048 samples per core.

LayerNorm: mean-subtraction is folded into the preceding matmul on the host
(W <- W - rowmean(W), b <- b - mean(b)) so the matmul output is already
centered and variance = mean(z^2). Per 512-tile: evict+bias (DVE), square
(GpSimd), column-sum matmul (PE, M=1, col-group packed 4 tiles into one
PSUM bank), rsqrt via DVE bit-trick + 2 bf16 Newton steps on the packed
[128,512] stat tile, replicate-matmul (PE, K=1, lhsT = gamma*sqrt(D) packed
at rows 0/32/64/96), normalize multiply (DVE), one ACT Silu with
per-partition beta bias. ACT Rsqrt/Reciprocal are banned (accuracy), and
Exp/Ln live in a different ACT table set than Silu, so the gate softmax
normalization uses a DVE bit-trick reciprocal.

Device output rows are {strain, tensile, softplus(gap)} as [3, 32768] fp32
per core; the host computes yield = tensile - softplus and assembles
[262144, 3].
"""

import sys
import numpy as np
from contextlib import ExitStack

sys.path.insert(0, "/opt/trn_rl_repo")

import concourse.bacc as bacc
import concourse.tile as tile
from concourse import mybir
from concourse.bass_utils import run_bass_kernel_spmd
import concourse.bass as bass

F32 = mybir.dt.float32
BF16 = mybir.dt.bfloat16
U32 = mybir.dt.uint32
AF = mybir.ActivationFunctionType
ALU = mybir.AluOpType

N_CORES = 8
B_TOTAL = 262144
B_CORE = B_TOTAL // N_CORES       # 32768
FD = 512                          # samples per PSUM tile
NT = 4                            # sub-tiles per super-tile
SUP = NT * FD                     # 2048 samples per super-tile
N_SUP = B_CORE // SUP             # 16 super-tiles per core
EPS = 1e-5
RSQRT_MAGIC = 0x5F3759DF
RECIP_MAGIC = 0x7EF477D5

_cached = {}


def _bf16(a):
    import ml_dtypes
    return np.ascontiguousarray(np.asarray(a, dtype=np.float32).astype(ml_dtypes.bfloat16))


def _col(a):
    return np.ascontiguousarray(np.asarray(a, np.float32).reshape(-1, 1))


def _prep_params(params):
    def _conv(v):
        if isinstance(v, dict):
            return {k: _conv(vv) for k, vv in v.items()}
        return np.asarray(v, np.float64)

    p = _conv(dict(params))
    ins = {}

    def ln_lin(name, w, b, g, beta):
        D = w.shape[1]
        wc = w - w.mean(axis=1, keepdims=True)
        bc = b - b.mean()
        ins[name + "_w"] = _bf16(wc)
        ins[name + "_bc"] = _col(bc)
        grp = np.zeros((128, D))
        for j in range(NT):
            grp[32 * j, :] = g * np.sqrt(D)
        ins[name + "_gr"] = _bf16(grp)
        ins[name + "_beta"] = _col(beta)

    def lin(name, w, b):
        ins[name + "_w"] = _bf16(w)
        ins[name + "_b"] = _col(b)

    ln_lin("stem1", p["stem_l1"]["w"], p["stem_l1"]["b"],
           p["stem_ln1"]["g"], p["stem_ln1"]["b"])
    ln_lin("stem2", p["stem_l2"]["w"], p["stem_l2"]["b"],
           p["stem_ln2"]["g"], p["stem_ln2"]["b"])
    for bi, bn in enumerate(["block1", "block2"]):
        blk = p[bn]
        ln_lin(f"b{bi}f1", blk["fc1"]["w"], blk["fc1"]["b"],
               blk["ln1"]["g"], blk["ln1"]["b"])
        ln_lin(f"b{bi}f2", blk["fc2"]["w"], blk["fc2"]["b"],
               blk["ln2"]["g"], blk["ln2"]["b"])
    lin("g1", p["gate_l1"]["w"], p["gate_l1"]["b"])
    lin("g2", p["gate_l2"]["w"], p["gate_l2"]["b"])
    e = p["experts"]
    for ei in range(6):
        ln_lin(f"e{ei}w1", e["w1"][ei], e["b1"][ei], e["lng"][ei], e["lnb"][ei])
        lin(f"e{ei}w2", e["w2"][ei], e["b2"][ei])
        lin(f"e{ei}w3", e["w3"][ei], e["b3"][ei])
    ln_lin("post", p["post_l"]["w"], p["post_l"]["b"],
           p["post_ln"]["g"], p["post_ln"]["b"])
    lin("shared", p["shared"]["w"], p["shared"]["b"])

    ins["g3_w"] = _bf16(p["gate_l3"]["w"])
    g3b = np.zeros((128, 1), np.float32)
    for j in range(NT):
        g3b[32 * j:32 * j + 6, 0] = p["gate_l3"]["b"]
    ins["g3_bpack"] = g3b

    hs, ht, hg = p["head_strain"], p["head_tensile"], p["yield_gap"]
    ins["h1a_w"] = _bf16(np.concatenate([hs["l1"]["w"], ht["l1"]["w"]], axis=1))
    ins["h1a_b"] = _col(np.concatenate([hs["l1"]["b"], ht["l1"]["b"]]))
    ins["h1b_w"] = _bf16(hg["l1"]["w"])
    ins["h1b_b"] = _col(hg["l1"]["b"])
    l2a = np.zeros((128, 64))
    l2a[:64, :32] = hs["l2"]["w"]
    l2a[64:, 32:] = ht["l2"]["w"]
    ins["h2a_w"] = _bf16(l2a)
    ins["h2b_w"] = _bf16(hg["l2"]["w"])
    ins["h2_b"] = _col(np.concatenate([hs["l2"]["b"], ht["l2"]["b"],
                                       hg["l2"]["b"]]))
    l3 = np.zeros((96, 3))
    l3[:32, 0] = hs["l3"]["w"][:, 0]
    l3[32:64, 1] = ht["l3"]["w"][:, 0]
    l3[64:, 2] = hg["l3"]["w"][:, 0]
    ins["h3_w"] = _bf16(l3)
    h3b = np.zeros((128, 1), np.float32)
    for j in range(NT):
        h3b[32 * j + 0, 0] = hs["l3"]["b"][0]
        h3b[32 * j + 1, 0] = ht["l3"]["b"][0]
        h3b[32 * j + 2, 0] = hg["l3"]["b"][0]
    ins["h3_bpack"] = h3b

    ins["ones"] = _bf16(np.ones((128, 128)))
    return ins


def _build(param_shapes):
    nc = bacc.Bacc()
    x_d = nc.dram_tensor("x", [64, B_CORE], BF16, kind="ExternalInput")
    out_d = nc.dram_tensor("out", [3, B_CORE], F32, kind="ExternalOutput")
    w_d = {}
    for name, arr in param_shapes.items():
        dt = BF16 if arr.dtype != np.float32 else F32
        w_d[name] = nc.dram_tensor(name, list(arr.shape), dt, kind="ExternalInput")

    with tile.TileContext(nc) as tc, ExitStack() as ctx:
        wp = ctx.enter_context(tc.tile_pool(name="wp", bufs=1))
        a1 = ctx.enter_context(tc.tile_pool(name="a1", bufs=1))
        a2 = ctx.enter_context(tc.tile_pool(name="a2", bufs=2))
        a3 = ctx.enter_context(tc.tile_pool(name="a3", bufs=3))
        stat = ctx.enter_context(tc.tile_pool(name="stat", bufs=1))
        zp = ctx.enter_context(tc.tile_pool(name="zp", bufs=2, space="PSUM"))
        rp = ctx.enter_context(tc.tile_pool(name="rp", bufs=2, space="PSUM"))
        vp = ctx.enter_context(tc.tile_pool(name="vp", bufs=2, space="PSUM"))
        mp = ctx.enter_context(tc.tile_pool(name="mp", bufs=2, space="PSUM"))

        W = {}
        for name, arr in param_shapes.items():
            dt = BF16 if arr.dtype != np.float32 else F32
            t = wp.tile(list(arr.shape), dt, tag=name)
            nc.sync.dma_start(out=t[:], in_=w_d[name][:])
            W[name] = t

        JS = [slice(j * FD, (j + 1) * FD) for j in range(NT)]

        def rsqrt_chain(vpack_ps, d_eps):
            ve = stat.tile([128, FD], F32, tag="ve")
            nc.vector.tensor_scalar(ve[:], vpack_ps[:], float(d_eps), None,
                                    ALU.add)
            i1 = stat.tile([128, FD], U32, tag="i1")
            nc.vector.tensor_scalar(i1[:], ve[:].bitcast(U32), 1, None,
                                    ALU.logical_shift_right)
            sd = stat.tile([128, FD], F32, tag="sd")
            nc.vector.tensor_scalar(sd[:].bitcast(U32), i1[:], -1, RSQRT_MAGIC,
                                    ALU.mult, ALU.add)
            sdb = stat.tile([128, FD], BF16, tag="sdb")
            nc.vector.tensor_copy(sdb[:], sd[:])
            web = stat.tile([128, FD], BF16, tag="web")
            nc.vector.tensor_copy(web[:], ve[:])
            cur = sdb
            for it in range(1):
                a = stat.tile([128, FD], BF16, tag=f"nA{it}")
                nc.vector.tensor_tensor(a[:], cur[:], cur[:], ALU.mult)
                b = stat.tile([128, FD], BF16, tag=f"nB{it}")
                nc.vector.tensor_tensor(b[:], a[:], web[:], ALU.mult)
                c = stat.tile([128, FD], BF16, tag=f"nC{it}")
                nc.vector.tensor_scalar(c[:], b[:], -0.5, 1.5, ALU.mult,
                                        ALU.add)
                r = stat.tile([128, FD], BF16, tag=f"nR{it}")
                nc.vector.tensor_tensor(r[:], cur[:], c[:], ALU.mult)
                cur = r
            return cur

        def recip_chain(spack_ps):
            sd = stat.tile([128, FD], F32, tag="rsd")
            nc.vector.tensor_scalar(sd[:].bitcast(U32), spack_ps[:].bitcast(U32),
                                    -1, RECIP_MAGIC, ALU.mult, ALU.add)
            sb = stat.tile([128, FD], BF16, tag="rsb")
            nc.vector.tensor_copy(sb[:], sd[:])
            wb = stat.tile([128, FD], BF16, tag="rwb")
            nc.vector.tensor_copy(wb[:], spack_ps[:])
            cur = sb
            for it in range(1):
                d = stat.tile([128, FD], BF16, tag=f"rD{it}")
                nc.vector.tensor_tensor(d[:], wb[:], cur[:], ALU.mult)
                e = stat.tile([128, FD], BF16, tag=f"rE{it}")
                nc.vector.tensor_scalar(e[:], d[:], -1.0, 2.0, ALU.mult,
                                        ALU.add)
                r = stat.tile([128, FD], BF16, tag=f"rR{it}")
                nc.vector.tensor_tensor(r[:], cur[:], e[:], ALU.mult)
                cur = r
            return cur

        def ln_layer(name, insup, din, dout, opool, otag, res=None):
            w, bc = W[name + "_w"], W[name + "_bc"]
            gr, beta = W[name + "_gr"], W[name + "_beta"]
            zsup = a2.tile([dout, SUP], BF16, tag="ln_z")
            sqsup = a2.tile([dout, SUP], BF16, tag="ln_sq")
            vpack = vp.tile([128, FD], F32, tag="vpack")
            for j in range(NT):
                z = zp.tile([128, FD], F32, tag="z")
                nc.tensor.matmul(z[:dout, :], w[:, :], insup[:, JS[j]],
                                 start=True, stop=True)
                nc.vector.tensor_scalar(zsup[:, JS[j]], z[:dout, :], bc[:, :],
                                        None, ALU.add)
                nc.gpsimd.tensor_tensor(sqsup[:, JS[j]], zsup[:, JS[j]],
                                        zsup[:, JS[j]], ALU.mult)
                nc.tensor.matmul(vpack[32 * j:32 * j + 1, :],
                                 W["ones"][:dout, 0:1], sqsup[:, JS[j]],
                                 start=True, stop=True,
                                 tile_position=(0, 32 * j))
            inv = rsqrt_chain(vpack, dout * EPS)
            tsup = a2.tile([dout, SUP], BF16, tag="ln_t")
            for j in range(NT):
                ir = rp.tile([128, FD], F32, tag="ir")
                nc.tensor.matmul(ir[:dout, :], gr[32 * j:32 * j + 1, :],
                                 inv[32 * j:32 * j + 1, :],
                                 start=True, stop=True,
                                 tile_position=(32 * j, 0))
                nc.vector.tensor_tensor(tsup[:, JS[j]], zsup[:, JS[j]],
                                        ir[:dout, :], ALU.mult)
            outsup = opool.tile([dout, SUP], BF16, tag=otag)
            if res is None:
                nc.scalar.activation(outsup[:], tsup[:], AF.Silu,
                                     bias=beta[:, :])
            else:
                rsup = a1.tile([dout, SUP], BF16, tag="ln_r")
                nc.gpsimd.tensor_tensor(rsup[:], tsup[:], res[:], ALU.add)
                nc.scalar.activation(outsup[:], rsup[:], AF.Silu,
                                     bias=beta[:, :])
            return outsup

        def lin_silu(name, insup, din, dout, opool, otag):
            w, b = W[name + "_w"], W[name + "_b"]
            outsup = opool.tile([dout, SUP], BF16, tag=otag)
            for j in range(NT):
                z = zp.tile([128, FD], F32, tag="z")
                nc.tensor.matmul(z[:dout, :], w[:, :], insup[:, JS[j]],
                                 start=True, stop=True)
                nc.scalar.activation(outsup[:, JS[j]], z[:dout, :], AF.Silu,
                                     bias=b[:, :])
            return outsup

        def body(isup):
            xoff = isup * SUP
            xs = a2.tile([64, SUP], BF16, tag="xs")
            nc.sync.dma_start(out=xs[:], in_=x_d[:, bass.ds(xoff, SUP)])

            t = ln_layer("stem1", xs, 64, 128, a3, "ln_o")
            h0 = ln_layer("stem2", t, 128, 96, a1, "hres0")
            t = ln_layer("b0f1", h0, 96, 128, a3, "ln_o")
            h1 = ln_layer("b0f2", t, 128, 96, a1, "hres1", res=h0)
            t = ln_layer("b1f1", h1, 96, 128, a3, "ln_o")
            h = ln_layer("b1f2", t, 128, 96, a1, "h", res=h1)

            # ---- gate ----
            g1 = lin_silu("g1", h, 96, 64, a1, "g1_o")
            g2 = lin_silu("g2", g1, 64, 32, a1, "g2_o")
            logit = mp.tile([128, FD], F32, tag="mp")
            nc.vector.memset(logit[:], 0.0)
            for j in range(NT):
                nc.tensor.matmul(logit[32 * j:32 * j + 6, :], W["g3_w"][:, :],
                                 g2[:, JS[j]], start=True, stop=True,
                                 tile_position=(0, 32 * j))
            expp = a1.tile([128, FD], BF16, tag="expp")
            nc.scalar.activation(expp[:], logit[:], AF.Exp,
                                 bias=W["g3_bpack"][:, :])
            spack = mp.tile([128, FD], F32, tag="mp")
            nc.vector.memset(spack[:], 0.0)
            for j in range(NT):
                nc.tensor.matmul(spack[32 * j:32 * j + 1, :],
                                 W["ones"][32 * j:32 * j + 6, 0:1],
                                 expp[32 * j:32 * j + 6, :],
                                 start=True, stop=True,
                                 tile_position=(32 * j, 32 * j))
            rcp = recip_chain(spack)
            # repack exp rows so each expert's gate row sits 32-aligned
            epks = []
            for ei in range(6):
                epk = a1.tile([128, FD], BF16, tag=f"epk{ei}")
                src = expp[:].rearrange("(a b) f -> a b f", b=32)[:, ei, :]
                dst = epk[:].rearrange("(a b) f -> a b f", b=32)[:, 0, :]
                nc.sync.dma_start(out=dst, in_=src)
                epks.append(epk)

            # ---- experts + moe ----
            macc = a1.tile([96, SUP], BF16, tag="macc")
            for ei in range(6):
                u = ln_layer(f"e{ei}w1", h, 96, 96, a3, "ew_u")
                u = lin_silu(f"e{ei}w2", u, 96, 96, a3, "ew_u")
                u = lin_silu(f"e{ei}w3", u, 96, 96, a3, "ew_u")
                term = macc if ei == 0 else a2.tile([96, SUP], BF16, tag="mt")
                for j in range(NT):
                    gr_ps = rp.tile([128, FD], F32, tag="ir")
                    nc.tensor.matmul(gr_ps[:96, :],
                                     W["ones"][32 * j:32 * j + 1, :96],
                                     epks[ei][32 * j:32 * j + 1, :],
                                     start=True, stop=True,
                                     tile_position=(32 * j, 0))
                    nc.vector.tensor_tensor(term[:, JS[j]], u[:, JS[j]],
                                            gr_ps[:96, :], ALU.mult)
                if ei > 0:
                    nc.gpsimd.tensor_tensor(macc[:], macc[:], term[:], ALU.add)

            moe = a1.tile([96, SUP], BF16, tag="moe")
            for j in range(NT):
                rs_ps = rp.tile([128, FD], F32, tag="ir")
                nc.tensor.matmul(rs_ps[:96, :],
                                 W["ones"][32 * j:32 * j + 1, :96],
                                 rcp[32 * j:32 * j + 1, :],
                                 start=True, stop=True,
                                 tile_position=(32 * j, 0))
                nc.vector.tensor_tensor(moe[:, JS[j]], macc[:, JS[j]],
                                        rs_ps[:96, :], ALU.mult)
            pre = a3.tile([96, SUP], BF16, tag="ln_o")
            nc.gpsimd.tensor_tensor(pre[:], moe[:], h[:], ALU.add)

            # ---- post / shared / heads ----
            feat = ln_layer("post", pre, 96, 96, a3, "ln_o")
            sh = lin_silu("shared", feat, 96, 64, a1, "sh_o")
            h1a = lin_silu("h1a", sh, 64, 128, a1, "h1a_o")
            h1b = lin_silu("h1b", sh, 64, 64, a1, "h1b_o")
            hh2 = a1.tile([96, SUP], BF16, tag="hh2")
            for j in range(NT):
                z = zp.tile([128, FD], F32, tag="z")
                nc.tensor.matmul(z[:64, :], W["h2a_w"][:, :], h1a[:, JS[j]],
                                 start=True, stop=True)
                nc.tensor.matmul(z[64:96, :], W["h2b_w"][:, :], h1b[:, JS[j]],
                                 start=True, stop=True, tile_position=(0, 64))
                nc.scalar.activation(hh2[:, JS[j]], z[:96, :], AF.Silu,
                                     bias=W["h2_b"][:, :])
            l3p = mp.tile([128, FD], F32, tag="mp")
            nc.vector.memset(l3p[:], 0.0)
            for j in range(NT):
                nc.tensor.matmul(l3p[32 * j:32 * j + 3, :], W["h3_w"][:, :],
                                 hh2[:, JS[j]], start=True, stop=True,
                                 tile_position=(0, 32 * j))
            sb3 = stat.tile([128, FD], F32, tag="sb3")
            nc.vector.tensor_scalar(sb3[:], l3p[:], W["h3_bpack"][:, :],
                                    None, ALU.add)
            ex3 = stat.tile([128, FD], F32, tag="ex3")
            nc.scalar.activation(ex3[:], sb3[:], AF.Exp)
            sp3 = stat.tile([128, FD], F32, tag="sp3")
            nc.scalar.activation(sp3[:], ex3[:], AF.Ln, bias=1.0)
            for j in range(NT):
                nc.sync.dma_start(out=out_d[0:2, bass.ds(xoff + j * FD, FD)],
                                  in_=sb3[32 * j:32 * j + 2, :])
                nc.sync.dma_start(out=out_d[2:3, bass.ds(xoff + j * FD, FD)],
                                  in_=sp3[32 * j + 2:32 * j + 3, :])

        with tc.For_i(0, N_SUP // 2, 1) as iv:
            body(iv * 2)
            body(iv * 2 + 1)

    nc.finalize()
    return nc


def kernel(x, params):
    import ml_dtypes
    x = np.asarray(x, np.float32)
    ins = _prep_params(params)

    if "net" not in _cached:
        _cached["net"] = _build(ins)
    nc = _cached["net"]

    xT = np.ascontiguousarray(x.T).astype(ml_dtypes.bfloat16)  # [64, B]
    in_maps = []
    for c in range(N_CORES):
        m = dict(ins)
        m["x"] = np.ascontiguousarray(xT[:, c * B_CORE:(c + 1) * B_CORE])
        in_maps.append(m)

    res = run_bass_kernel_spmd(nc, in_maps, core_ids=list(range(N_CORES)),
                               trace=False)
    outs = []
    for c in range(N_CORES):
        o = res.results[c]["out"]
        y = np.empty((B_CORE, 3), np.float32)
        y[:, 0] = o[0]
        y[:, 1] = o[1]
        y[:, 2] = o[1] - o[2]
        outs.append(y)
    return np.concatenate(outs, axis=0)
